# revision 43
# baseline (speedup 1.0000x reference)
"""Trainium2 Bass kernel for a 2-layer GraphNetwork (gnn_message_passing).

Strategy ("one-mode", all-bf16):
  - 16 graphs across 8 cores (2/core, paired big-with-small to balance
    load); every edge's receiver is core-local, so all segment
    reductions stay on-core. [16,128] outputs gathered on host.
  - ALL matmuls run with tile_size (128,128): small contractions are
    zero-padded to 128 rows (cost is free-dim-bound, so padding rows
    are free). Any tiling-config change (row-banded, col-banded,
    DoubleRow) costs a ~200-300ns pipeline drain on this part AND keeps
    the PE HAM clock at 1.2GHz; a uniform (128,128) stream runs warm at
    2.4GHz, which beats fp8-DoubleRow's 2x/instruction.
  - The e1 edge-layer matmul also produces the e2 globals/bias init in
    the same instruction (extra stationary rows: ones -> be1|be2,
    graph-one-hots -> globals projections), FD=384.
  - Segment sums are one-hot-selector matmuls; one-hots built on host.
  - agg transposes are plain matmuls against an identity moving operand
    (out = lhsT.T @ I), avoiding transpose-mode switches.
  - Two-stage software pipeline across edge pairs and across tiles so
    the statically-scheduled PE stream never waits on DVE/Act
    evacuations; PSUM evacuations are merged full-bank ops balanced
    across ScalarE and VectorE.
  - fp32 PSUM everywhere; final projection fp32.
"""

import numpy as np
import ml_dtypes

import concourse.bass as bass
import concourse.tile as tile_mod
from concourse import tile
from concourse.bass_utils import run_bass_kernel_spmd
from concourse.vector_clock import ScopedClock

mybir = bass.mybir

N_NODES, N_EDGES, N_GRAPHS = 20000, 320000, 16
F_NODE, F_EDGE, F_GLOB = 64, 32, 16
N_CORES = 8
GPC = N_GRAPHS // N_CORES  # graphs per core = 2

BF16 = mybir.dt.bfloat16
F32 = mybir.dt.float32
FP8 = mybir.dt.float8e4
npbf16 = ml_dtypes.bfloat16
npfp8 = mybir.dt.np(FP8)
DR = mybir.MatmulPerfMode.DoubleRow

# ---------------------------------------------------------------------------
# Workaround: CoreV3 codegen rejects the TileContext final drain when it
# carries more than one semaphore wait. Split the waits across extra no-ops.
_MAX_WAITS = 1
_ENGINE_WAIT_LIMIT = 1
_SPLIT_ENGINES = None


def _split_excess_waits(nc):
    global _SPLIT_ENGINES
    if _SPLIT_ENGINES is None:
        ET = mybir.EngineType
        _SPLIT_ENGINES = {ET.PE, ET.Activation, ET.DVE, ET.SP, ET.Pool}
    ctr = [0]
    for bass_bb in nc.bb_map.values():
        bb = bass_bb.bb
        il = bb.instructions
        out = []
        changed = False
        for inst in il:
            si = inst.sync_info
            waits = list(si.on_wait) if (si and si.on_wait) else []
            if len(waits) > _ENGINE_WAIT_LIMIT and inst.engine in _SPLIT_ENGINES:
                head, keep = waits[:-_ENGINE_WAIT_LIMIT], waits[-_ENGINE_WAIT_LIMIT:]
                for i in range(0, len(head), _ENGINE_WAIT_LIMIT):
                    nop = mybir.InstNoOp(name=f"waitsplit-{ctr[0]}", ins=[], outs=[])
                    ctr[0] += 1
                    nop.engine = inst.engine
                    nop.sync_info = mybir.SyncInfo(
                        on_wait=head[i : i + _ENGINE_WAIT_LIMIT], on_update=[]
                    )
                    nc.register_instruction(nop, overwrite=True)
                    out.append(nop)
                inst.sync_info = mybir.SyncInfo(
                    on_wait=keep, on_update=list(si.on_update or [])
                )
                changed = True
            out.append(inst)
        if changed:
            bb.instructions = out


def _split_drain_and_barrier(self, tick_clock, wait_clock):
    nc = self.nc
    _split_excess_waits(nc)
    drain_inst = nc.sync.drain()
    wait_clock.add_sem_waits(
        drain_inst.ins, ScopedClock({None: tick_clock.global_clock})
    )
    mi = drain_inst.ins
    waits = list(mi.sync_info.on_wait) if (mi.sync_info and mi.sync_info.on_wait) else []
    if len(waits) > _MAX_WAITS:
        upd = list(mi.sync_info.on_update) if mi.sync_info.on_update else []
        mi.sync_info = mybir.SyncInfo(on_wait=waits[:_MAX_WAITS], on_update=upd)
        for i in range(_MAX_WAITS, len(waits), _MAX_WAITS):
            nop = nc.sync.nop(nofuse=True)
            nop.ins.sync_info = mybir.SyncInfo(
                on_wait=waits[i : i + _MAX_WAITS], on_update=[]
            )
    nc.all_engine_barrier()
    assert self.sems is not None
    popped = nc._tile_sem_poison_stack.pop()
    assert popped is self._sem_poison
    nc.clear_and_free_semaphores(list(self.sems.allocated().values()))
    nc.all_engine_barrier()


tile_mod.TileContext._drain_and_barrier = _split_drain_and_barrier


# ---------------------------------------------------------------------------
# Host-side graph partitioning / layout


def _pack_core(node_ids, degs, nt, cap_e):
    order = np.argsort(-degs, kind="stable")
    tiles_n = [[] for _ in range(nt)]
    tile_ncnt = np.zeros(nt, np.int64)
    tile_ecnt = np.zeros(nt, np.int64)
    for j in order:
        cand = np.where(tile_ncnt < 128)[0]
        if len(cand) == 0:
            return None
        t = cand[np.argmin(tile_ecnt[cand])]
        tiles_n[t].append(node_ids[j])
        tile_ncnt[t] += 1
        tile_ecnt[t] += degs[j]
    if (tile_ecnt > cap_e).any():
        return None
    return [np.array(t, dtype=np.int64) for t in tiles_n]


def _prepare(inputs):
    nf = np.asarray(inputs["node_feats"], np.float32)
    ef = np.asarray(inputs["edge_feats"], np.float32)
    glob = np.asarray(inputs["globals_"], np.float32)
    recv = np.asarray(inputs["receivers"]).astype(np.int64)
    ngraph = np.asarray(inputs["node_graph"]).astype(np.int64)

    cnt = np.bincount(recv, minlength=N_NODES).astype(np.int64)
    egraph = ngraph[recv]
    ncnt_g = np.bincount(ngraph, minlength=N_GRAPHS)
    ecnt_g = np.bincount(egraph, minlength=N_GRAPHS)

    # pair heavy graphs with light ones to balance nodes across cores
    order = np.argsort(ncnt_g, kind="stable")
    graph_core = np.zeros(N_GRAPHS, np.int64)
    graph_slot = np.zeros(N_GRAPHS, np.int64)
    core_graphs = []
    for c in range(N_CORES):
        ga, gb = int(order[c]), int(order[N_GRAPHS - 1 - c])
        graph_core[ga] = c
        graph_slot[ga] = 0
        graph_core[gb] = c
        graph_slot[gb] = 1
        core_graphs.append((ga, gb))

    node_core = graph_core[ngraph]
    edge_core = graph_core[egraph]

    core_nodes = [np.where(node_core == c)[0] for c in range(N_CORES)]
    NT = int(max((len(cn) + 127) // 128 for cn in core_nodes))

    packs = None
    K0 = max(1, int(max(np.bincount(edge_core, minlength=N_CORES)) + NT * 128 - 1)
             // (NT * 128))
    if K0 % 2:
        K0 += 1
    for k0 in range(K0, K0 + 13, 2):
        trial = []
        ok = True
        for c in range(N_CORES):
            p = _pack_core(core_nodes[c], cnt[core_nodes[c]], NT, k0 * 128)
            if p is None:
                ok = False
                break
            trial.append(p)
        if ok:
            packs, K0 = trial, k0
            break
    assert packs is not None, "bin packing failed"

    NPAD = NT * 128
    EPAD = NT * K0 * 128
    NPAIR = NT * K0 // 2

    # --- shared weights (core-independent parts)
    We1T = np.asarray(inputs["We1"], np.float32).T  # [32, 256]
    be1 = np.asarray(inputs["be1"], np.float32)
    be2 = np.asarray(inputs["be2"], np.float32)
    bn2 = np.asarray(inputs["bn2"], np.float32)

    We1TKb = np.zeros((128, 256), np.float32)
    We1TKb[0:32] = We1T
    We1TKb[32] = be1

    We2T = np.asarray(inputs["We2"], np.float32).T  # [256, 128]
    We2DR = np.concatenate([We2T[:128], We2T[128:]], axis=1)  # [128, 256]

    Wn1T = np.asarray(inputs["Wn1"], np.float32).T  # [64, 256]
    Wn1TK = np.zeros((128, 256), np.float32)
    Wn1TK[0:64] = Wn1T
    bn1c = np.asarray(inputs["bn1"], np.float32).reshape(2, 128).T.copy()  # [128,2]

    Win1T = np.asarray(inputs["Win1"], np.float32).T  # [256, 256]
    Win1DR = np.zeros((128, 512), np.float32)
    for s in range(2):
        for i in range(2):
            Win1DR[:, 256 * s + 128 * i : 256 * s + 128 * i + 128] = \
                Win1T[128 * i : 128 * i + 128, 128 * s : 128 * s + 128]

    Wn2T = np.asarray(inputs["Wn2"], np.float32).T
    Wn2DR = np.concatenate([Wn2T[:128], Wn2T[128:]], axis=1)
    Win2T = np.asarray(inputs["Win2"], np.float32).T

    Wg2T = np.asarray(inputs["Wg2"], np.float32).T  # [16, 128]
    Wng2T = np.asarray(inputs["Wng2"], np.float32).T

    w_np = {
        "We1TKb": We1TKb.astype(npbf16),
        "We2DR": We2DR.astype(npbf16),
        "Wn1TK": Wn1TK.astype(npbf16),
        "bn1c": bn1c,
        "Win1DR": Win1DR.astype(npbf16),
        "Wn2DR": Wn2DR.astype(npbf16),
        "Win2f8": Win2T.astype(npbf16),
        "WgnT": np.asarray(inputs["Wgn"], np.float32).T.copy(),
        "WgeT": np.asarray(inputs["Wge"], np.float32).T.copy(),
        "WggT": np.asarray(inputs["Wgg"], np.float32).T.copy(),
        "bgr": np.asarray(inputs["bg"], np.float32)[None, :].copy(),
        "ones2": np.ones((1, 2), np.float32),
        "ident": np.eye(128, dtype=npbf16),
        "ident2": np.eye(2, dtype=np.float32),
    }

    slot_of_node = np.full(N_NODES, -1, np.int64)
    tile_of_node = np.full(N_NODES, -1, np.int64)
    in_maps = []
    for c in range(N_CORES):
        for t in range(NT):
            ids = packs[c][t]
            slot_of_node[ids] = t * 128 + np.arange(len(ids))
            tile_of_node[ids] = t

        # ---- edges: assign slots (grouped by receiver tile)
        eidx = np.where(edge_core == c)[0]
        et = tile_of_node[recv[eidx]]
        eorder = np.argsort(et, kind="stable")
        eidx = eidx[eorder]
        et = et[eorder]
        counts = np.bincount(et, minlength=NT)
        starts = np.concatenate([[0], np.cumsum(counts)[:-1]])
        off_in = np.arange(len(eidx)) - np.repeat(starts, counts)
        dst = et * (K0 * 128) + off_in
        assert (counts <= K0 * 128).all()

        eg_loc = graph_slot[egraph[eidx]]
        # eftM: one [128,128] column-block per chunk.
        # rows 0:32 feats, 32 ones, 33 isg0, 34 isg1, rest zero.
        eftM = np.zeros((128, EPAD), np.float32)
        eftM[0:32, dst] = ef[eidx].T
        eftM[32, dst] = 1.0
        eftM[33, dst] = (eg_loc == 0)
        eftM[34, dst] = (eg_loc == 1)

        # one-hot selectors, bf16, chunk-major: ohb[e, chunk*128 + n]
        sel = np.full(EPAD, -1, np.int64)
        sel[dst] = slot_of_node[recv[eidx]] % 128
        oh = np.zeros((EPAD, 128), np.float32)
        vmask = sel >= 0
        oh[np.where(vmask)[0], sel[vmask]] = 1.0
        oh2 = (
            oh.reshape(NT * K0, 128, 128)
            .transpose(1, 0, 2)
            .reshape(128, EPAD)
        )

        # merged e1 + e2-init stationary weights (per-core globals)
        ga, gb = core_graphs[c]
        gl = np.stack([glob[ga], glob[gb]])  # [2, 16]
        gp = gl @ Wg2T  # [2, 128]
        We1Kx = np.zeros((128, 384), np.float32)
        We1Kx[0:32, 0:256] = We1T
        We1Kx[32, 0:256] = be1
        We1Kx[32, 256:384] = be2
        We1Kx[33, 256:384] = gp[0]
        We1Kx[34, 256:384] = gp[1]

        gn = gl @ Wng2T
        gnaugK = np.zeros((128, 128), np.float32)
        gnaugK[0:2] = gn
        gnaugK[2] = bn2

        # ---- nodes
        slot_node = np.full(NPAD, -1, np.int64)
        for t in range(NT):
            ids = packs[c][t]
            slot_node[t * 128 : t * 128 + len(ids)] = ids
        valid = slot_node >= 0
        sn = np.where(valid, slot_node, 0)

        nftK = np.zeros((128, NPAD), np.float32)
        nftK[0:64][:, valid] = nf[sn[valid]].T

        ng_loc = graph_slot[ngraph[sn]]
        nhotK = np.zeros((128, NPAD), np.float32)
        nhotK[0] = valid * (ng_loc == 0)
        nhotK[1] = valid * (ng_loc == 1)
        nhotK[2] = valid * 1.0

        invc2 = np.zeros((NPAD, 1), np.float32)
        invc2[valid, 0] = 1.0 / np.maximum(cnt[sn[valid]], 1)
        invc2 = invc2.reshape(NT, 128).T.copy()  # [128, NT]

        # zero-padded pool weight stationaries: cols 0:2 carry the weights
        poolw2 = np.zeros((NPAD, 256), np.float32)
        for g in range(GPC):
            gid = core_graphs[c][g]
            m = valid & (ng_loc == g)
            poolw2[m, g] = 1.0 / max(ncnt_g[gid], 1)
            poolw2[m, 128 + g] = cnt[sn[m]] / max(ecnt_g[gid], 1)

        globT = gl.T.copy()  # [16, 2]

        m = {
            "eftM": eftM.astype(npbf16),
            "oh2": oh2.astype(npbf16),
            "We1Kx": We1Kx.astype(npbf16),
            "gnaugK": gnaugK.astype(npbf16),
            "nftK": nftK.astype(npbf16),
            "nhotK": nhotK.astype(npbf16),
            "invc2": invc2,
            "poolw2": poolw2.astype(npbf16),
            "globT": globT,
        }
        m.update(w_np)
        in_maps.append(m)

    return in_maps, NT, K0, [core_graphs[c] for c in range(N_CORES)]


# ---------------------------------------------------------------------------
# Device program (identical on all cores)


def _build(NT, K0):
    Relu = mybir.ActivationFunctionType.Relu
    Copy = mybir.ActivationFunctionType.Copy

    nc = bass.Bass()
    NPAD = NT * 128
    EPAD = NT * K0 * 128
    NPAIR = NT * K0 // 2
    PPT = K0 // 2  # pairs per tile
    CW = K0 * 128  # eftM cols per tile
    OW = PPT * 256  # oh2 cols per tile

    d_eftM = nc.dram_tensor("eftM", [128, EPAD], BF16, kind="ExternalInput")
    d_oh2 = nc.dram_tensor("oh2", [128, EPAD], BF16, kind="ExternalInput")
    d_We1Kx = nc.dram_tensor("We1Kx", [128, 384], BF16, kind="ExternalInput")
    d_gnaugK = nc.dram_tensor("gnaugK", [128, 128], BF16, kind="ExternalInput")
    d_nftK = nc.dram_tensor("nftK", [128, NPAD], BF16, kind="ExternalInput")
    d_nhotK = nc.dram_tensor("nhotK", [128, NPAD], BF16, kind="ExternalInput")
    d_invc2 = nc.dram_tensor("invc2", [128, NT], F32, kind="ExternalInput")
    d_poolw2 = nc.dram_tensor("poolw2", [NPAD, 256], BF16, kind="ExternalInput")
    d_globT = nc.dram_tensor("globT", [16, 2], F32, kind="ExternalInput")

    d_We1TKb = nc.dram_tensor("We1TKb", [128, 256], BF16, kind="ExternalInput")
    d_We2DR = nc.dram_tensor("We2DR", [128, 256], BF16, kind="ExternalInput")
    d_Wn1TK = nc.dram_tensor("Wn1TK", [128, 256], BF16, kind="ExternalInput")
    d_bn1c = nc.dram_tensor("bn1c", [128, 2], F32, kind="ExternalInput")
    d_Win1DR = nc.dram_tensor("Win1DR", [128, 512], BF16, kind="ExternalInput")
    d_Wn2DR = nc.dram_tensor("Wn2DR", [128, 256], BF16, kind="ExternalInput")
    d_Win2f8 = nc.dram_tensor("Win2f8", [128, 128], BF16, kind="ExternalInput")
    d_WgnT = nc.dram_tensor("WgnT", [128, 128], F32, kind="ExternalInput")
    d_WgeT = nc.dram_tensor("WgeT", [128, 128], F32, kind="ExternalInput")
    d_WggT = nc.dram_tensor("WggT", [16, 128], F32, kind="ExternalInput")
    d_bgr = nc.dram_tensor("bgr", [1, 128], F32, kind="ExternalInput")
    d_ones2 = nc.dram_tensor("ones2", [1, 2], F32, kind="ExternalInput")
    d_ident = nc.dram_tensor("ident", [128, 128], BF16, kind="ExternalInput")
    d_ident2 = nc.dram_tensor("ident2", [2, 2], F32, kind="ExternalInput")

    d_out = nc.dram_tensor("out", [128, 2], F32, kind="ExternalOutput")

    def r3(ap, blk):
        return ap.rearrange("p (a b) -> p a b", a=2, b=blk)

    with tile.TileContext(nc) as tc:
        with tc.tile_pool(name="wp", bufs=1) as wp:
            def wtile(dram, shape, dt):
                t = wp.tile(shape, dt, tag=dram.name)
                nc.sync.dma_start(t[:], dram[:])
                return t

            We1Kx = wtile(d_We1Kx, [128, 384], BF16)
            We1TKb = wtile(d_We1TKb, [128, 256], BF16)
            We2DR = wtile(d_We2DR, [128, 256], BF16)
            gnaugK = wtile(d_gnaugK, [128, 128], BF16)
            Wn1TK = wtile(d_Wn1TK, [128, 256], BF16)
            bn1c = wtile(d_bn1c, [128, 2], F32)
            Win1DR = wtile(d_Win1DR, [128, 512], BF16)
            Wn2DR = wtile(d_Wn2DR, [128, 256], BF16)
            Win2f8 = wtile(d_Win2f8, [128, 128], BF16)
            WgnT = wtile(d_WgnT, [128, 128], F32)
            WgeT = wtile(d_WgeT, [128, 128], F32)
            WggT = wtile(d_WggT, [16, 128], F32)
            bgr = wtile(d_bgr, [1, 128], F32)
            ones2 = wtile(d_ones2, [1, 2], F32)
            ident = wtile(d_ident, [128, 128], BF16)
            ident2 = wtile(d_ident2, [2, 2], F32)
            globT = wtile(d_globT, [16, 2], F32)
            invc2 = wtile(d_invc2, [128, NT], F32)

            with tc.tile_pool(name="ep", bufs=3) as ep, \
                 tc.tile_pool(name="esb", bufs=4) as esb, \
                 tc.tile_pool(name="nsb", bufs=3) as nsb, \
                 tc.tile_pool(name="psME", bufs=2, space=bass.MemorySpace.PSUM) as psME, \
                 tc.tile_pool(name="psMO", bufs=2, space=bass.MemorySpace.PSUM) as psMO, \
                 tc.tile_pool(name="psT1", bufs=1, space=bass.MemorySpace.PSUM) as psT1, \
                 tc.tile_pool(name="psAgg", bufs=1, space=bass.MemorySpace.PSUM) as psAgg, \
                 tc.tile_pool(name="psA", bufs=1, space=bass.MemorySpace.PSUM) as psA, \
                 tc.tile_pool(name="psB", bufs=1, space=bass.MemorySpace.PSUM) as psB:

                # node-phase PSUM shares two banks; all groups are closed
                # per tile, so interleavings stay legal.
                bankA = psA.tile([128, 512], F32, tag="bankA")
                pn1 = bankA[:, 0:256]
                bankB = psB.tile([128, 512], F32, tag="bankB")
                ptrT = bankB[:, 0:384]
                pn2 = bankB[:, 384:512]
                accP = None

                state = {"pair": None, "node": None, "accP": None}

                def emit_pair_tail(p):
                    """pe2 + evacuations + aggregation for a pair."""
                    mgE, mgO, e1bf, ef2, pagg, e0, e1s, oht, j, ppt_ = p[:10]
                    nc.tensor.matmul(mgE[:, 256:384], e1bf[:, 0:128],
                                     We2DR[:, 0:128], start=False, stop=False)
                    nc.tensor.matmul(mgE[:, 256:384], e1bf[:, 128:256],
                                     We2DR[:, 128:256], start=False, stop=True)
                    nc.tensor.matmul(mgO[:, 256:384], e1bf[:, 256:384],
                                     We2DR[:, 0:128], start=False, stop=False)
                    nc.tensor.matmul(mgO[:, 256:384], e1bf[:, 384:512],
                                     We2DR[:, 128:256], start=False, stop=True)
                    # one full-width relu evac per chunk (DVE)
                    nc.vector.tensor_scalar_max(ef2[:, 0:384], mgE[:], 0.0)
                    nc.vector.tensor_scalar_max(ef2[:, 384:768], mgO[:], 0.0)
                    nc.tensor.matmul(pagg[:], oht[:, e0], ef2[:, 0:384],
                                     start=(j == 0), stop=False)
                    nc.tensor.matmul(pagg[:], oht[:, e1s], ef2[:, 384:768],
                                     start=False, stop=(j == ppt_ - 1))

                def emit_node_rest(nd):
                    """node phase after aggsb: transposes, n1, n2, pools."""
                    aggsb, nftt, nht, pw = nd
                    nc.tensor.matmul(ptrT[:, 0:128], aggsb[:, 0:128], ident[:],
                                     start=True, stop=True)
                    nc.tensor.matmul(ptrT[:, 128:256], aggsb[:, 128:256], ident[:],
                                     start=True, stop=True)
                    nc.tensor.matmul(ptrT[:, 256:384], aggsb[:, 256:384], ident[:],
                                     start=True, stop=True)
                    aggT = nsb.tile([128, 384], BF16, tag="aggT")
                    nc.scalar.activation(aggT[:], ptrT, Relu)

                    for s in range(2):
                        sc = slice(s * 128, (s + 1) * 128)
                        nc.tensor.matmul(pn1[:, sc], Wn1TK[:, sc], nftt[:],
                                         start=True, stop=False)
                        nc.tensor.matmul(pn1[:, sc],
                                         Win1DR[:, 256 * s : 256 * s + 128],
                                         aggT[:, 0:128], start=False, stop=False)
                        nc.tensor.matmul(pn1[:, sc],
                                         Win1DR[:, 256 * s + 128 : 256 * s + 256],
                                         aggT[:, 128:256], start=False, stop=True)
                    n1bf = nsb.tile([128, 256], BF16, tag="n1bf")
                    for s in range(2):
                        sc = slice(s * 128, (s + 1) * 128)
                        nc.scalar.activation(n1bf[:, sc], pn1[:, sc], Relu,
                                             bias=bn1c[:, s : s + 1])

                    nc.tensor.matmul(pn2, nht[:], gnaugK[:], start=True, stop=False)
                    nc.tensor.matmul(pn2, n1bf[:, 0:128], Wn2DR[:, 0:128],
                                     start=False, stop=False)
                    nc.tensor.matmul(pn2, n1bf[:, 128:256], Wn2DR[:, 128:256],
                                     start=False, stop=False)
                    nc.tensor.matmul(pn2, aggT[:, 256:384], Win2f8[:],
                                     start=False, stop=True)
                    n2bf = nsb.tile([128, 128], BF16, tag="n2bf")
                    nc.scalar.activation(n2bf[:], pn2, Relu)

                    ppt = bankA[:, 256:512]
                    nc.tensor.matmul(ppt[:, 0:128], pw[:, 0:128], n2bf[:],
                                     start=True, stop=True)
                    nc.tensor.matmul(ppt[:, 128:256], pw[:, 128:256],
                                     aggsb[:, 256:384], start=True, stop=True)
                    accP_new = nsb.tile([2, 256], F32, tag="accP")
                    if state["accP"] is None:
                        nc.vector.tensor_copy(accP_new[:], ppt[0:2, :])
                    else:
                        nc.vector.tensor_tensor(accP_new[:], state["accP"][:],
                                                ppt[0:2, :],
                                                op=mybir.AluOpType.add)
                    state["accP"] = accP_new

                for t in range(NT):
                    eftt = ep.tile([128, CW], BF16, tag="eftt")
                    nc.sync.dma_start(eftt[:], d_eftM[:, t * CW : (t + 1) * CW])
                    oht = ep.tile([128, CW], BF16, tag="oht")
                    nc.sync.dma_start(oht[:], d_oh2[:, t * CW : (t + 1) * CW])
                    nftt = ep.tile([128, 128], BF16, tag="nftt")
                    nc.sync.dma_start(nftt[:], d_nftK[:, t * 128 : (t + 1) * 128])
                    nht = ep.tile([128, 128], BF16, tag="nht")
                    nc.sync.dma_start(nht[:], d_nhotK[:, t * 128 : (t + 1) * 128])
                    pw = ep.tile([128, 256], BF16, tag="pw")
                    nc.sync.dma_start(pw[:], d_poolw2[t * 128 : (t + 1) * 128, :])

                    pagg = psAgg.tile([128, 384], F32, tag="pagg")

                    for j in range(PPT):
                        e0 = slice(2 * j * 128, 2 * j * 128 + 128)
                        e1s = slice((2 * j + 1) * 128, (2 * j + 1) * 128 + 128)
                        epr = slice(2 * j * 128, 2 * j * 128 + 256)

                        mgE = psME.tile([128, 384], F32, tag="mgE")
                        mgO = psMO.tile([128, 384], F32, tag="mgO")
                        e1T2 = psT1.tile([128, 512], F32, tag="e1T2")

                        # merged e1 + e2-init (FD=384), one per chunk
                        nc.tensor.matmul(mgE[:], eftt[:, e0], We1Kx[:],
                                         start=True, stop=False)
                        nc.tensor.matmul(mgO[:], eftt[:, e1s], We1Kx[:],
                                         start=True, stop=False)
                        # e1T blocks for the pair (FD=256 each)
                        nc.tensor.matmul(e1T2[:, 0:256], We1TKb[:, 0:128],
                                         eftt[:, epr], start=True, stop=True)
                        nc.tensor.matmul(e1T2[:, 256:512], We1TKb[:, 128:256],
                                         eftt[:, epr], start=True, stop=True)

                        # evacuate e1 feat-major -> e1bf (Act: relu, reshuffle
                        # blk-major -> chunk-major while casting)
                        ef2 = esb.tile([128, 768], BF16, tag="ef2")
                        e1bf = esb.tile([128, 512], BF16, tag="e1bf")
                        src = e1T2[:].rearrange("p (b c e) -> p c b e", b=2, c=2, e=128)
                        dst = e1bf[:].rearrange("p (c b e) -> p c b e", c=2, b=2, e=128)
                        nc.scalar.activation(dst, src, Relu)

                        # software pipeline: finish the PREVIOUS pair now that
                        # this pair's first-stage matmuls are in the stream.
                        if state["pair"] is not None:
                            pt = state["pair"]
                            emit_pair_tail(pt)
                            if pt[10] != t:
                                # that was the previous tile's last pair:
                                # kick off its node phase
                                aggsb = nsb.tile([128, 384], BF16, tag="aggsb")
                                nc.scalar.activation(
                                    aggsb[:], pt[4][:], Copy,
                                    scale=invc2[:, pt[10] : pt[10] + 1])
                                state["node"] = (aggsb, pt[11], pt[12], pt[13])
                        state["pair"] = (mgE, mgO, e1bf, ef2, pagg, e0, e1s,
                                         oht, j, PPT, t, nftt, nht, pw)

                        # deferred node phase of the previous tile
                        if j == 2 and state["node"] is not None:
                            emit_node_rest(state["node"])
                            state["node"] = None

                # drain: last pair + last node phase
                pt = state["pair"]
                emit_pair_tail(pt)
                aggsb = nsb.tile([128, 384], BF16, tag="aggsb")
                nc.scalar.activation(aggsb[:], pt[4][:], Copy,
                                     scale=invc2[:, pt[10] : pt[10] + 1])
                emit_node_rest((aggsb, pt[11], pt[12], pt[13]))
                accP = state["accP"]

                # ----------------- final projection -----------------
                ptr2 = bankB[:, 0:4]
                nc.tensor.matmul(ptr2[:, 0:2], accP[:, 0:128], ident2[:],
                                 start=True, stop=True)
                nc.tensor.matmul(ptr2[:, 2:4], accP[:, 128:256], ident2[:],
                                 start=True, stop=True)
                nt2 = nsb.tile([128, 4], F32, tag="nt2")
                nc.scalar.activation(nt2[:], ptr2, Copy)

                pout = bankA[:, 256:258]
                nc.tensor.matmul(pout, WgnT[:], nt2[:, 0:2], start=True, stop=False)
                nc.tensor.matmul(pout, WgeT[:], nt2[:, 2:4], start=False, stop=False)
                nc.tensor.matmul(pout, WggT[:], globT[:], start=False, stop=False)
                nc.tensor.matmul(pout, bgr[:], ones2[:], start=False, stop=True)
                outsb = nsb.tile([128, 2], F32, tag="outsb")
                nc.scalar.activation(outsb[:], pout, Copy)
                nc.sync.dma_start(d_out[:], outsb[:])

    return nc


_CACHE = {}


def _get_nc(NT, K0):
    key = (NT, K0)
    if key not in _CACHE:
        _CACHE[key] = _build(NT, K0)
    return _CACHE[key]


def _run(inputs, trace=False):
    in_maps, NT, K0, core_graphs = _prepare(inputs)
    nc = _get_nc(NT, K0)
    res = run_bass_kernel_spmd(nc, in_maps, list(range(N_CORES)), trace=trace)
    out = np.zeros((N_GRAPHS, 128), np.float32)
    for c in range(N_CORES):
        r = np.asarray(res.results[c]["out"], np.float32)
        ga, gb = core_graphs[c]
        out[ga] = r[:, 0]
        out[gb] = r[:, 1]
    return out, res


def kernel(**inputs):
    out, _ = _run(inputs, trace=False)
    return out


def kernel_traced(**inputs):
    return _run(inputs, trace=True)


# revision 47
# speedup vs baseline: 1.0141x; 1.0141x over previous
"""Trainium2 Bass kernel for a 2-layer GraphNetwork (gnn_message_passing).

Strategy ("one-mode", all-bf16):
  - 16 graphs across 8 cores (2/core, paired big-with-small to balance
    load); every edge's receiver is core-local, so all segment
    reductions stay on-core. [16,128] outputs gathered on host.
  - ALL matmuls run with tile_size (128,128): small contractions are
    zero-padded to 128 rows (cost is free-dim-bound, so padding rows
    are free). Any tiling-config change (row-banded, col-banded,
    DoubleRow) costs a ~200-300ns pipeline drain on this part AND keeps
    the PE HAM clock at 1.2GHz; a uniform (128,128) stream runs warm at
    2.4GHz, which beats fp8-DoubleRow's 2x/instruction.
  - The e1 edge-layer matmul also produces the e2 globals/bias init in
    the same instruction (extra stationary rows: ones -> be1|be2,
    graph-one-hots -> globals projections), FD=384.
  - Segment sums are one-hot-selector matmuls; one-hots built on host.
  - agg transposes are plain matmuls against an identity moving operand
    (out = lhsT.T @ I), avoiding transpose-mode switches.
  - Two-stage software pipeline across edge pairs and across tiles so
    the statically-scheduled PE stream never waits on DVE/Act
    evacuations; PSUM evacuations are merged full-bank ops balanced
    across ScalarE and VectorE.
  - fp32 PSUM everywhere; final projection fp32.
"""

import numpy as np
import ml_dtypes

import concourse.bass as bass
import concourse.tile as tile_mod
from concourse import tile
from concourse.bass_utils import run_bass_kernel_spmd
from concourse.vector_clock import ScopedClock

mybir = bass.mybir

N_NODES, N_EDGES, N_GRAPHS = 20000, 320000, 16
F_NODE, F_EDGE, F_GLOB = 64, 32, 16
N_CORES = 8
GPC = N_GRAPHS // N_CORES  # graphs per core = 2

BF16 = mybir.dt.bfloat16
F32 = mybir.dt.float32
FP8 = mybir.dt.float8e4
npbf16 = ml_dtypes.bfloat16
npfp8 = mybir.dt.np(FP8)
DR = mybir.MatmulPerfMode.DoubleRow

# ---------------------------------------------------------------------------
# Workaround: CoreV3 codegen rejects the TileContext final drain when it
# carries more than one semaphore wait. Split the waits across extra no-ops.
_MAX_WAITS = 1
_ENGINE_WAIT_LIMIT = 1
_SPLIT_ENGINES = None


def _split_excess_waits(nc):
    global _SPLIT_ENGINES
    if _SPLIT_ENGINES is None:
        ET = mybir.EngineType
        _SPLIT_ENGINES = {ET.PE, ET.Activation, ET.DVE, ET.SP, ET.Pool}
    ctr = [0]
    for bass_bb in nc.bb_map.values():
        bb = bass_bb.bb
        il = bb.instructions
        out = []
        changed = False
        for inst in il:
            si = inst.sync_info
            waits = list(si.on_wait) if (si and si.on_wait) else []
            if len(waits) > _ENGINE_WAIT_LIMIT and inst.engine in _SPLIT_ENGINES:
                head, keep = waits[:-_ENGINE_WAIT_LIMIT], waits[-_ENGINE_WAIT_LIMIT:]
                for i in range(0, len(head), _ENGINE_WAIT_LIMIT):
                    nop = mybir.InstNoOp(name=f"waitsplit-{ctr[0]}", ins=[], outs=[])
                    ctr[0] += 1
                    nop.engine = inst.engine
                    nop.sync_info = mybir.SyncInfo(
                        on_wait=head[i : i + _ENGINE_WAIT_LIMIT], on_update=[]
                    )
                    nc.register_instruction(nop, overwrite=True)
                    out.append(nop)
                inst.sync_info = mybir.SyncInfo(
                    on_wait=keep, on_update=list(si.on_update or [])
                )
                changed = True
            out.append(inst)
        if changed:
            bb.instructions = out


def _split_drain_and_barrier(self, tick_clock, wait_clock):
    nc = self.nc
    _split_excess_waits(nc)
    drain_inst = nc.sync.drain()
    wait_clock.add_sem_waits(
        drain_inst.ins, ScopedClock({None: tick_clock.global_clock})
    )
    mi = drain_inst.ins
    waits = list(mi.sync_info.on_wait) if (mi.sync_info and mi.sync_info.on_wait) else []
    if len(waits) > _MAX_WAITS:
        upd = list(mi.sync_info.on_update) if mi.sync_info.on_update else []
        mi.sync_info = mybir.SyncInfo(on_wait=waits[:_MAX_WAITS], on_update=upd)
        for i in range(_MAX_WAITS, len(waits), _MAX_WAITS):
            nop = nc.sync.nop(nofuse=True)
            nop.ins.sync_info = mybir.SyncInfo(
                on_wait=waits[i : i + _MAX_WAITS], on_update=[]
            )
    nc.all_engine_barrier()
    assert self.sems is not None
    popped = nc._tile_sem_poison_stack.pop()
    assert popped is self._sem_poison
    nc.clear_and_free_semaphores(list(self.sems.allocated().values()))
    nc.all_engine_barrier()


tile_mod.TileContext._drain_and_barrier = _split_drain_and_barrier


# ---------------------------------------------------------------------------
# Host-side graph partitioning / layout


def _pack_core(node_ids, degs, nt, cap_e):
    order = np.argsort(-degs, kind="stable")
    tiles_n = [[] for _ in range(nt)]
    tile_ncnt = np.zeros(nt, np.int64)
    tile_ecnt = np.zeros(nt, np.int64)
    for j in order:
        cand = np.where(tile_ncnt < 128)[0]
        if len(cand) == 0:
            return None
        t = cand[np.argmin(tile_ecnt[cand])]
        tiles_n[t].append(node_ids[j])
        tile_ncnt[t] += 1
        tile_ecnt[t] += degs[j]
    if (tile_ecnt > cap_e).any():
        return None
    return [np.array(t, dtype=np.int64) for t in tiles_n]


def _prepare(inputs):
    nf = np.asarray(inputs["node_feats"], np.float32)
    ef = np.asarray(inputs["edge_feats"], np.float32)
    glob = np.asarray(inputs["globals_"], np.float32)
    recv = np.asarray(inputs["receivers"]).astype(np.int64)
    ngraph = np.asarray(inputs["node_graph"]).astype(np.int64)

    cnt = np.bincount(recv, minlength=N_NODES).astype(np.int64)
    egraph = ngraph[recv]
    ncnt_g = np.bincount(ngraph, minlength=N_GRAPHS)
    ecnt_g = np.bincount(egraph, minlength=N_GRAPHS)

    # pair heavy graphs with light ones to balance nodes across cores
    order = np.argsort(ncnt_g, kind="stable")
    graph_core = np.zeros(N_GRAPHS, np.int64)
    graph_slot = np.zeros(N_GRAPHS, np.int64)
    core_graphs = []
    for c in range(N_CORES):
        ga, gb = int(order[c]), int(order[N_GRAPHS - 1 - c])
        graph_core[ga] = c
        graph_slot[ga] = 0
        graph_core[gb] = c
        graph_slot[gb] = 1
        core_graphs.append((ga, gb))

    node_core = graph_core[ngraph]
    edge_core = graph_core[egraph]

    core_nodes = [np.where(node_core == c)[0] for c in range(N_CORES)]
    NT = int(max((len(cn) + 127) // 128 for cn in core_nodes))

    packs = None
    K0 = max(1, int(max(np.bincount(edge_core, minlength=N_CORES)) + NT * 128 - 1)
             // (NT * 128))
    if K0 % 2:
        K0 += 1
    for k0 in range(K0, K0 + 13, 2):
        trial = []
        ok = True
        for c in range(N_CORES):
            p = _pack_core(core_nodes[c], cnt[core_nodes[c]], NT, k0 * 128)
            if p is None:
                ok = False
                break
            trial.append(p)
        if ok:
            packs, K0 = trial, k0
            break
    assert packs is not None, "bin packing failed"

    NPAD = NT * 128
    EPAD = NT * K0 * 128
    NPAIR = NT * K0 // 2

    # --- shared weights (core-independent parts)
    We1T = np.asarray(inputs["We1"], np.float32).T  # [32, 256]
    be1 = np.asarray(inputs["be1"], np.float32)
    be2 = np.asarray(inputs["be2"], np.float32)
    bn2 = np.asarray(inputs["bn2"], np.float32)

    We1TKb = np.zeros((128, 256), np.float32)
    We1TKb[0:32] = We1T
    We1TKb[32] = be1

    We2T = np.asarray(inputs["We2"], np.float32).T  # [256, 128]
    We2DR = np.concatenate([We2T[:128], We2T[128:]], axis=1)  # [128, 256]

    Wn1T = np.asarray(inputs["Wn1"], np.float32).T  # [64, 256]
    Wn1TK = np.zeros((128, 256), np.float32)
    Wn1TK[0:64] = Wn1T
    Wn1TK[64] = np.asarray(inputs["bn1"], np.float32)  # bias via ones-row
    bn1c = np.asarray(inputs["bn1"], np.float32).reshape(2, 128).T.copy()  # [128,2]

    Win1T = np.asarray(inputs["Win1"], np.float32).T  # [256, 256]
    Win1DR = np.zeros((128, 512), np.float32)
    for s in range(2):
        for i in range(2):
            Win1DR[:, 256 * s + 128 * i : 256 * s + 128 * i + 128] = \
                Win1T[128 * i : 128 * i + 128, 128 * s : 128 * s + 128]

    Wn2T = np.asarray(inputs["Wn2"], np.float32).T
    Wn2DR = np.concatenate([Wn2T[:128], Wn2T[128:]], axis=1)
    Win2T = np.asarray(inputs["Win2"], np.float32).T

    Wg2T = np.asarray(inputs["Wg2"], np.float32).T  # [16, 128]
    Wng2T = np.asarray(inputs["Wng2"], np.float32).T

    w_np = {
        "We1TKb": We1TKb.astype(npbf16),
        "We2DR": We2DR.astype(npbf16),
        "Wn1TK": Wn1TK.astype(npbf16),
        "bn1c": bn1c,
        "Win1DR": Win1DR.astype(npbf16),
        "Wn2DR": Wn2DR.astype(npbf16),
        "Win2f8": Win2T.astype(npbf16),
        "WgnT": np.asarray(inputs["Wgn"], np.float32).T.copy(),
        "WgeT": np.asarray(inputs["Wge"], np.float32).T.copy(),
        "WggT": np.asarray(inputs["Wgg"], np.float32).T.copy(),
        "bgr": np.asarray(inputs["bg"], np.float32)[None, :].copy(),
        "ones2": np.ones((1, 2), np.float32),
        "ident": np.eye(128, dtype=npbf16),
        "ident2": np.eye(2, dtype=np.float32),
    }

    slot_of_node = np.full(N_NODES, -1, np.int64)
    tile_of_node = np.full(N_NODES, -1, np.int64)
    in_maps = []
    for c in range(N_CORES):
        for t in range(NT):
            ids = packs[c][t]
            slot_of_node[ids] = t * 128 + np.arange(len(ids))
            tile_of_node[ids] = t

        # ---- edges: assign slots (grouped by receiver tile)
        eidx = np.where(edge_core == c)[0]
        et = tile_of_node[recv[eidx]]
        eorder = np.argsort(et, kind="stable")
        eidx = eidx[eorder]
        et = et[eorder]
        counts = np.bincount(et, minlength=NT)
        starts = np.concatenate([[0], np.cumsum(counts)[:-1]])
        off_in = np.arange(len(eidx)) - np.repeat(starts, counts)
        dst = et * (K0 * 128) + off_in
        assert (counts <= K0 * 128).all()

        eg_loc = graph_slot[egraph[eidx]]
        # eftM: one [128,128] column-block per chunk.
        # rows 0:32 feats, 32 ones, 33 isg0, 34 isg1, rest zero.
        eftM = np.zeros((128, EPAD), np.float32)
        eftM[0:32, dst] = ef[eidx].T
        eftM[32, dst] = 1.0
        eftM[33, dst] = (eg_loc == 0)
        eftM[34, dst] = (eg_loc == 1)

        # one-hot selectors, bf16, chunk-major: ohb[e, chunk*128 + n]
        sel = np.full(EPAD, -1, np.int64)
        sel[dst] = slot_of_node[recv[eidx]] % 128
        oh = np.zeros((EPAD, 128), np.float32)
        vmask = sel >= 0
        oh[np.where(vmask)[0], sel[vmask]] = 1.0
        oh2 = (
            oh.reshape(NT * K0, 128, 128)
            .transpose(1, 0, 2)
            .reshape(128, EPAD)
        )

        # merged e1 + e2-init stationary weights (per-core globals)
        ga, gb = core_graphs[c]
        gl = np.stack([glob[ga], glob[gb]])  # [2, 16]
        gp = gl @ Wg2T  # [2, 128]
        We1Kx = np.zeros((128, 384), np.float32)
        We1Kx[0:32, 0:256] = We1T
        We1Kx[32, 0:256] = be1
        We1Kx[32, 256:384] = be2
        We1Kx[33, 256:384] = gp[0]
        We1Kx[34, 256:384] = gp[1]

        gn = gl @ Wng2T
        gnaugK = np.zeros((128, 128), np.float32)
        gnaugK[0:2] = gn
        gnaugK[2] = bn2

        # ---- nodes
        slot_node = np.full(NPAD, -1, np.int64)
        for t in range(NT):
            ids = packs[c][t]
            slot_node[t * 128 : t * 128 + len(ids)] = ids
        valid = slot_node >= 0
        sn = np.where(valid, slot_node, 0)

        nftK = np.zeros((128, NPAD), np.float32)
        nftK[0:64][:, valid] = nf[sn[valid]].T
        nftK[64] = valid * 1.0  # ones-row pairs with the bn1 row in Wn1TK

        ng_loc = graph_slot[ngraph[sn]]
        nhotK = np.zeros((128, NPAD), np.float32)
        nhotK[0] = valid * (ng_loc == 0)
        nhotK[1] = valid * (ng_loc == 1)
        nhotK[2] = valid * 1.0

        invc2 = np.zeros((NPAD, 1), np.float32)
        invc2[valid, 0] = 1.0 / np.maximum(cnt[sn[valid]], 1)
        invc2 = invc2.reshape(NT, 128).T.copy()  # [128, NT]

        # zero-padded pool weight stationaries: cols 0:2 carry the weights
        poolw2 = np.zeros((NPAD, 256), np.float32)
        for g in range(GPC):
            gid = core_graphs[c][g]
            m = valid & (ng_loc == g)
            poolw2[m, g] = 1.0 / max(ncnt_g[gid], 1)
            poolw2[m, 128 + g] = cnt[sn[m]] / max(ecnt_g[gid], 1)

        globT = gl.T.copy()  # [16, 2]

        m = {
            "eftM": eftM.astype(npbf16),
            "oh2": oh2.astype(npbf16),
            "We1Kx": We1Kx.astype(npbf16),
            "gnaugK": gnaugK.astype(npbf16),
            "nftK": nftK.astype(npbf16),
            "nhotK": nhotK.astype(npbf16),
            "invc2": invc2,
            "poolw2": poolw2.astype(npbf16),
            "globT": globT,
        }
        m.update(w_np)
        in_maps.append(m)

    return in_maps, NT, K0, [core_graphs[c] for c in range(N_CORES)]


# ---------------------------------------------------------------------------
# Device program (identical on all cores)


def _build(NT, K0):
    Relu = mybir.ActivationFunctionType.Relu
    Copy = mybir.ActivationFunctionType.Copy

    nc = bass.Bass()
    NPAD = NT * 128
    EPAD = NT * K0 * 128
    NPAIR = NT * K0 // 2
    PPT = K0 // 2  # pairs per tile
    CW = K0 * 128  # eftM cols per tile
    OW = PPT * 256  # oh2 cols per tile

    d_eftM = nc.dram_tensor("eftM", [128, EPAD], BF16, kind="ExternalInput")
    d_oh2 = nc.dram_tensor("oh2", [128, EPAD], BF16, kind="ExternalInput")
    d_We1Kx = nc.dram_tensor("We1Kx", [128, 384], BF16, kind="ExternalInput")
    d_gnaugK = nc.dram_tensor("gnaugK", [128, 128], BF16, kind="ExternalInput")
    d_nftK = nc.dram_tensor("nftK", [128, NPAD], BF16, kind="ExternalInput")
    d_nhotK = nc.dram_tensor("nhotK", [128, NPAD], BF16, kind="ExternalInput")
    d_invc2 = nc.dram_tensor("invc2", [128, NT], F32, kind="ExternalInput")
    d_poolw2 = nc.dram_tensor("poolw2", [NPAD, 256], BF16, kind="ExternalInput")
    d_globT = nc.dram_tensor("globT", [16, 2], F32, kind="ExternalInput")

    d_We1TKb = nc.dram_tensor("We1TKb", [128, 256], BF16, kind="ExternalInput")
    d_We2DR = nc.dram_tensor("We2DR", [128, 256], BF16, kind="ExternalInput")
    d_Wn1TK = nc.dram_tensor("Wn1TK", [128, 256], BF16, kind="ExternalInput")
    d_bn1c = nc.dram_tensor("bn1c", [128, 2], F32, kind="ExternalInput")
    d_Win1DR = nc.dram_tensor("Win1DR", [128, 512], BF16, kind="ExternalInput")
    d_Wn2DR = nc.dram_tensor("Wn2DR", [128, 256], BF16, kind="ExternalInput")
    d_Win2f8 = nc.dram_tensor("Win2f8", [128, 128], BF16, kind="ExternalInput")
    d_WgnT = nc.dram_tensor("WgnT", [128, 128], F32, kind="ExternalInput")
    d_WgeT = nc.dram_tensor("WgeT", [128, 128], F32, kind="ExternalInput")
    d_WggT = nc.dram_tensor("WggT", [16, 128], F32, kind="ExternalInput")
    d_bgr = nc.dram_tensor("bgr", [1, 128], F32, kind="ExternalInput")
    d_ones2 = nc.dram_tensor("ones2", [1, 2], F32, kind="ExternalInput")
    d_ident = nc.dram_tensor("ident", [128, 128], BF16, kind="ExternalInput")
    d_ident2 = nc.dram_tensor("ident2", [2, 2], F32, kind="ExternalInput")

    d_out = nc.dram_tensor("out", [128, 2], F32, kind="ExternalOutput")

    def r3(ap, blk):
        return ap.rearrange("p (a b) -> p a b", a=2, b=blk)

    with tile.TileContext(nc) as tc:
        with tc.tile_pool(name="wp", bufs=1) as wp:
            def wtile(dram, shape, dt):
                t = wp.tile(shape, dt, tag=dram.name)
                nc.sync.dma_start(t[:], dram[:])
                return t

            We1Kx = wtile(d_We1Kx, [128, 384], BF16)
            We1TKb = wtile(d_We1TKb, [128, 256], BF16)
            We2DR = wtile(d_We2DR, [128, 256], BF16)
            gnaugK = wtile(d_gnaugK, [128, 128], BF16)
            Wn1TK = wtile(d_Wn1TK, [128, 256], BF16)
            bn1c = wtile(d_bn1c, [128, 2], F32)
            Win1DR = wtile(d_Win1DR, [128, 512], BF16)
            Wn2DR = wtile(d_Wn2DR, [128, 256], BF16)
            Win2f8 = wtile(d_Win2f8, [128, 128], BF16)
            WgnT = wtile(d_WgnT, [128, 128], F32)
            WgeT = wtile(d_WgeT, [128, 128], F32)
            WggT = wtile(d_WggT, [16, 128], F32)
            bgr = wtile(d_bgr, [1, 128], F32)
            ones2 = wtile(d_ones2, [1, 2], F32)
            ident = wtile(d_ident, [128, 128], BF16)
            ident2 = wtile(d_ident2, [2, 2], F32)
            globT = wtile(d_globT, [16, 2], F32)
            invc2 = wtile(d_invc2, [128, NT], F32)

            with tc.tile_pool(name="ep", bufs=3) as ep, \
                 tc.tile_pool(name="esb", bufs=4) as esb, \
                 tc.tile_pool(name="nsb", bufs=3) as nsb, \
                 tc.tile_pool(name="psME", bufs=2, space=bass.MemorySpace.PSUM) as psME, \
                 tc.tile_pool(name="psMO", bufs=2, space=bass.MemorySpace.PSUM) as psMO, \
                 tc.tile_pool(name="psT1", bufs=1, space=bass.MemorySpace.PSUM) as psT1, \
                 tc.tile_pool(name="psAgg", bufs=1, space=bass.MemorySpace.PSUM) as psAgg, \
                 tc.tile_pool(name="psA", bufs=1, space=bass.MemorySpace.PSUM) as psA, \
                 tc.tile_pool(name="psB", bufs=1, space=bass.MemorySpace.PSUM) as psB:

                # node-phase PSUM shares two banks; all groups are closed
                # per tile, so interleavings stay legal.
                bankA = psA.tile([128, 512], F32, tag="bankA")
                pn1 = bankA[:, 0:256]
                bankB = psB.tile([128, 512], F32, tag="bankB")
                ptrT = bankB[:, 0:384]
                pn2 = bankB[:, 384:512]
                accP = None

                state = {"pair": None, "node": None, "accP": None}

                def emit_pair_tail(p):
                    """pe2 + evacuations + aggregation for a pair."""
                    mgE, mgO, e1bf, ef2, pagg, e0, e1s, oht, j, ppt_ = p[:10]
                    nc.tensor.matmul(mgE[:, 256:384], e1bf[:, 0:128],
                                     We2DR[:, 0:128], start=False, stop=False)
                    nc.tensor.matmul(mgE[:, 256:384], e1bf[:, 128:256],
                                     We2DR[:, 128:256], start=False, stop=True)
                    nc.tensor.matmul(mgO[:, 256:384], e1bf[:, 256:384],
                                     We2DR[:, 0:128], start=False, stop=False)
                    nc.tensor.matmul(mgO[:, 256:384], e1bf[:, 384:512],
                                     We2DR[:, 128:256], start=False, stop=True)
                    # one full-width relu evac per chunk (DVE)
                    nc.vector.tensor_scalar_max(ef2[:, 0:384], mgE[:], 0.0)
                    nc.vector.tensor_scalar_max(ef2[:, 384:768], mgO[:], 0.0)
                    nc.tensor.matmul(pagg[:], oht[:, e0], ef2[:, 0:384],
                                     start=(j == 0), stop=False)
                    nc.tensor.matmul(pagg[:], oht[:, e1s], ef2[:, 384:768],
                                     start=False, stop=(j == ppt_ - 1))

                def emit_node_rest(nd):
                    """node phase after aggsb: transposes, n1, n2, pools."""
                    aggsb, nftt, nht, pw = nd
                    nc.tensor.matmul(ptrT[:, 0:128], aggsb[:, 0:128], ident[:],
                                     start=True, stop=True)
                    nc.tensor.matmul(ptrT[:, 128:256], aggsb[:, 128:256], ident[:],
                                     start=True, stop=True)
                    nc.tensor.matmul(ptrT[:, 256:384], aggsb[:, 256:384], ident[:],
                                     start=True, stop=True)
                    aggT = nsb.tile([128, 384], BF16, tag="aggT")
                    nc.scalar.activation(aggT[:], ptrT, Relu)

                    for s in range(2):
                        sc = slice(s * 128, (s + 1) * 128)
                        nc.tensor.matmul(pn1[:, sc], Wn1TK[:, sc], nftt[:],
                                         start=True, stop=False)
                        nc.tensor.matmul(pn1[:, sc],
                                         Win1DR[:, 256 * s : 256 * s + 128],
                                         aggT[:, 0:128], start=False, stop=False)
                        nc.tensor.matmul(pn1[:, sc],
                                         Win1DR[:, 256 * s + 128 : 256 * s + 256],
                                         aggT[:, 128:256], start=False, stop=True)
                    n1bf = nsb.tile([128, 256], BF16, tag="n1bf")
                    nc.scalar.activation(n1bf[:], pn1, Relu)

                    nc.tensor.matmul(pn2, nht[:], gnaugK[:], start=True, stop=False)
                    nc.tensor.matmul(pn2, n1bf[:, 0:128], Wn2DR[:, 0:128],
                                     start=False, stop=False)
                    nc.tensor.matmul(pn2, n1bf[:, 128:256], Wn2DR[:, 128:256],
                                     start=False, stop=False)
                    nc.tensor.matmul(pn2, aggT[:, 256:384], Win2f8[:],
                                     start=False, stop=True)
                    n2bf = nsb.tile([128, 128], BF16, tag="n2bf")
                    nc.scalar.activation(n2bf[:], pn2, Relu)

                    ppt = bankA[:, 256:512]
                    nc.tensor.matmul(ppt[:, 0:128], pw[:, 0:128], n2bf[:],
                                     start=True, stop=True)
                    nc.tensor.matmul(ppt[:, 128:256], pw[:, 128:256],
                                     aggsb[:, 256:384], start=True, stop=True)
                    accP_new = nsb.tile([2, 256], F32, tag="accP")
                    if state["accP"] is None:
                        nc.vector.tensor_copy(accP_new[:], ppt[0:2, :])
                    else:
                        nc.vector.tensor_tensor(accP_new[:], state["accP"][:],
                                                ppt[0:2, :],
                                                op=mybir.AluOpType.add)
                    state["accP"] = accP_new

                for t in range(NT):
                    eftt = ep.tile([128, CW], BF16, tag="eftt")
                    nc.sync.dma_start(eftt[:], d_eftM[:, t * CW : (t + 1) * CW])
                    oht = ep.tile([128, CW], BF16, tag="oht")
                    nc.sync.dma_start(oht[:], d_oh2[:, t * CW : (t + 1) * CW])
                    nftt = ep.tile([128, 128], BF16, tag="nftt")
                    nc.sync.dma_start(nftt[:], d_nftK[:, t * 128 : (t + 1) * 128])
                    nht = ep.tile([128, 128], BF16, tag="nht")
                    nc.sync.dma_start(nht[:], d_nhotK[:, t * 128 : (t + 1) * 128])
                    pw = ep.tile([128, 256], BF16, tag="pw")
                    nc.sync.dma_start(pw[:], d_poolw2[t * 128 : (t + 1) * 128, :])

                    pagg = psAgg.tile([128, 384], F32, tag="pagg")

                    for j in range(PPT):
                        e0 = slice(2 * j * 128, 2 * j * 128 + 128)
                        e1s = slice((2 * j + 1) * 128, (2 * j + 1) * 128 + 128)
                        epr = slice(2 * j * 128, 2 * j * 128 + 256)

                        mgE = psME.tile([128, 384], F32, tag="mgE")
                        mgO = psMO.tile([128, 384], F32, tag="mgO")
                        e1T2 = psT1.tile([128, 512], F32, tag="e1T2")

                        # merged e1 + e2-init (FD=384), one per chunk
                        nc.tensor.matmul(mgE[:], eftt[:, e0], We1Kx[:],
                                         start=True, stop=False)
                        nc.tensor.matmul(mgO[:], eftt[:, e1s], We1Kx[:],
                                         start=True, stop=False)
                        # e1T blocks for the pair (FD=256 each)
                        nc.tensor.matmul(e1T2[:, 0:256], We1TKb[:, 0:128],
                                         eftt[:, epr], start=True, stop=True)
                        nc.tensor.matmul(e1T2[:, 256:512], We1TKb[:, 128:256],
                                         eftt[:, epr], start=True, stop=True)

                        ef2 = esb.tile([128, 768], BF16, tag="ef2")
                        e1bf = esb.tile([128, 512], BF16, tag="e1bf")

                        # software pipeline: finish the PREVIOUS pair now that
                        # this pair's first-stage matmuls are in the stream.
                        # (aggsb is emitted before this pair's e1bf so the Act
                        # queue unblocks the next tile's pagg group promptly.)
                        if state["pair"] is not None:
                            pt = state["pair"]
                            emit_pair_tail(pt)
                            if pt[10] != t:
                                # that was the previous tile's last pair:
                                # kick off its node phase
                                aggsb = nsb.tile([128, 384], BF16, tag="aggsb")
                                nc.scalar.activation(
                                    aggsb[:], pt[4][:], Copy,
                                    scale=invc2[:, pt[10] : pt[10] + 1])
                                state["node"] = (aggsb, pt[11], pt[12], pt[13])

                        # evacuate e1 feat-major -> e1bf (Act: relu, reshuffle
                        # blk-major -> chunk-major while casting)
                        src = e1T2[:].rearrange("p (b c e) -> p c b e", b=2, c=2, e=128)
                        dst = e1bf[:].rearrange("p (c b e) -> p c b e", c=2, b=2, e=128)
                        nc.scalar.activation(dst, src, Relu)

                        state["pair"] = (mgE, mgO, e1bf, ef2, pagg, e0, e1s,
                                         oht, j, PPT, t, nftt, nht, pw)

                        # deferred node phase of the previous tile
                        if j == 2 and state["node"] is not None:
                            emit_node_rest(state["node"])
                            state["node"] = None

                # drain: last pair + last node phase
                pt = state["pair"]
                emit_pair_tail(pt)
                aggsb = nsb.tile([128, 384], BF16, tag="aggsb")
                nc.scalar.activation(aggsb[:], pt[4][:], Copy,
                                     scale=invc2[:, pt[10] : pt[10] + 1])
                emit_node_rest((aggsb, pt[11], pt[12], pt[13]))
                accP = state["accP"]

                # ----------------- final projection -----------------
                ptr2 = bankB[:, 0:4]
                nc.tensor.matmul(ptr2[:, 0:2], accP[:, 0:128], ident2[:],
                                 start=True, stop=True)
                nc.tensor.matmul(ptr2[:, 2:4], accP[:, 128:256], ident2[:],
                                 start=True, stop=True)
                nt2 = nsb.tile([128, 4], F32, tag="nt2")
                nc.scalar.activation(nt2[:], ptr2, Copy)

                pout = bankA[:, 256:258]
                nc.tensor.matmul(pout, WgnT[:], nt2[:, 0:2], start=True, stop=False)
                nc.tensor.matmul(pout, WgeT[:], nt2[:, 2:4], start=False, stop=False)
                nc.tensor.matmul(pout, WggT[:], globT[:], start=False, stop=False)
                nc.tensor.matmul(pout, bgr[:], ones2[:], start=False, stop=True)
                outsb = nsb.tile([128, 2], F32, tag="outsb")
                nc.scalar.activation(outsb[:], pout, Copy)
                nc.sync.dma_start(d_out[:], outsb[:])

    return nc


_CACHE = {}


def _get_nc(NT, K0):
    key = (NT, K0)
    if key not in _CACHE:
        _CACHE[key] = _build(NT, K0)
    return _CACHE[key]


def _run(inputs, trace=False):
    in_maps, NT, K0, core_graphs = _prepare(inputs)
    nc = _get_nc(NT, K0)
    res = run_bass_kernel_spmd(nc, in_maps, list(range(N_CORES)), trace=trace)
    out = np.zeros((N_GRAPHS, 128), np.float32)
    for c in range(N_CORES):
        r = np.asarray(res.results[c]["out"], np.float32)
        ga, gb = core_graphs[c]
        out[ga] = r[:, 0]
        out[gb] = r[:, 1]
    return out, res


def kernel(**inputs):
    out, _ = _run(inputs, trace=False)
    return out


def kernel_traced(**inputs):
    return _run(inputs, trace=True)


# revision 49
# speedup vs baseline: 1.0301x; 1.0158x over previous
"""Trainium2 Bass kernel for a 2-layer GraphNetwork (gnn_message_passing).

Strategy ("one-mode", all-bf16):
  - 16 graphs across 8 cores (2/core, paired big-with-small to balance
    load); every edge's receiver is core-local, so all segment
    reductions stay on-core. [16,128] outputs gathered on host.
  - ALL matmuls run with tile_size (128,128): small contractions are
    zero-padded to 128 rows (cost is free-dim-bound, so padding rows
    are free). Any tiling-config change (row-banded, col-banded,
    DoubleRow) costs a ~200-300ns pipeline drain on this part AND keeps
    the PE HAM clock at 1.2GHz; a uniform (128,128) stream runs warm at
    2.4GHz, which beats fp8-DoubleRow's 2x/instruction.
  - The e1 edge-layer matmul also produces the e2 globals/bias init in
    the same instruction (extra stationary rows: ones -> be1|be2,
    graph-one-hots -> globals projections), FD=384.
  - Segment sums are one-hot-selector matmuls; one-hots built on host.
  - agg transposes are plain matmuls against an identity moving operand
    (out = lhsT.T @ I), avoiding transpose-mode switches.
  - Two-stage software pipeline across edge pairs and across tiles so
    the statically-scheduled PE stream never waits on DVE/Act
    evacuations; PSUM evacuations are merged full-bank ops balanced
    across ScalarE and VectorE.
  - fp32 PSUM everywhere; final projection fp32.
"""

import numpy as np
import ml_dtypes

import concourse.bass as bass
import concourse.tile as tile_mod
from concourse import tile
from concourse.bass_utils import run_bass_kernel_spmd
from concourse.vector_clock import ScopedClock

mybir = bass.mybir

N_NODES, N_EDGES, N_GRAPHS = 20000, 320000, 16
F_NODE, F_EDGE, F_GLOB = 64, 32, 16
N_CORES = 8
GPC = N_GRAPHS // N_CORES  # graphs per core = 2

BF16 = mybir.dt.bfloat16
F32 = mybir.dt.float32
FP8 = mybir.dt.float8e4
npbf16 = ml_dtypes.bfloat16
npfp8 = mybir.dt.np(FP8)
DR = mybir.MatmulPerfMode.DoubleRow

# ---------------------------------------------------------------------------
# Workaround: CoreV3 codegen rejects the TileContext final drain when it
# carries more than one semaphore wait. Split the waits across extra no-ops.
_MAX_WAITS = 1
_ENGINE_WAIT_LIMIT = 1
_SPLIT_ENGINES = None


def _split_excess_waits(nc):
    global _SPLIT_ENGINES
    if _SPLIT_ENGINES is None:
        ET = mybir.EngineType
        _SPLIT_ENGINES = {ET.PE, ET.Activation, ET.DVE, ET.SP, ET.Pool}
    ctr = [0]
    for bass_bb in nc.bb_map.values():
        bb = bass_bb.bb
        il = bb.instructions
        out = []
        changed = False
        for inst in il:
            si = inst.sync_info
            waits = list(si.on_wait) if (si and si.on_wait) else []
            if len(waits) > _ENGINE_WAIT_LIMIT and inst.engine in _SPLIT_ENGINES:
                head, keep = waits[:-_ENGINE_WAIT_LIMIT], waits[-_ENGINE_WAIT_LIMIT:]
                for i in range(0, len(head), _ENGINE_WAIT_LIMIT):
                    nop = mybir.InstNoOp(name=f"waitsplit-{ctr[0]}", ins=[], outs=[])
                    ctr[0] += 1
                    nop.engine = inst.engine
                    nop.sync_info = mybir.SyncInfo(
                        on_wait=head[i : i + _ENGINE_WAIT_LIMIT], on_update=[]
                    )
                    nc.register_instruction(nop, overwrite=True)
                    out.append(nop)
                inst.sync_info = mybir.SyncInfo(
                    on_wait=keep, on_update=list(si.on_update or [])
                )
                changed = True
            out.append(inst)
        if changed:
            bb.instructions = out


def _split_drain_and_barrier(self, tick_clock, wait_clock):
    nc = self.nc
    _split_excess_waits(nc)
    drain_inst = nc.sync.drain()
    wait_clock.add_sem_waits(
        drain_inst.ins, ScopedClock({None: tick_clock.global_clock})
    )
    mi = drain_inst.ins
    waits = list(mi.sync_info.on_wait) if (mi.sync_info and mi.sync_info.on_wait) else []
    if len(waits) > _MAX_WAITS:
        upd = list(mi.sync_info.on_update) if mi.sync_info.on_update else []
        mi.sync_info = mybir.SyncInfo(on_wait=waits[:_MAX_WAITS], on_update=upd)
        for i in range(_MAX_WAITS, len(waits), _MAX_WAITS):
            nop = nc.sync.nop(nofuse=True)
            nop.ins.sync_info = mybir.SyncInfo(
                on_wait=waits[i : i + _MAX_WAITS], on_update=[]
            )
    nc.all_engine_barrier()
    assert self.sems is not None
    popped = nc._tile_sem_poison_stack.pop()
    assert popped is self._sem_poison
    nc.clear_and_free_semaphores(list(self.sems.allocated().values()))
    nc.all_engine_barrier()


tile_mod.TileContext._drain_and_barrier = _split_drain_and_barrier


# ---------------------------------------------------------------------------
# Host-side graph partitioning / layout


def _pack_core(node_ids, degs, nt, cap_e):
    order = np.argsort(-degs, kind="stable")
    tiles_n = [[] for _ in range(nt)]
    tile_ncnt = np.zeros(nt, np.int64)
    tile_ecnt = np.zeros(nt, np.int64)
    for j in order:
        cand = np.where(tile_ncnt < 128)[0]
        if len(cand) == 0:
            return None
        t = cand[np.argmin(tile_ecnt[cand])]
        tiles_n[t].append(node_ids[j])
        tile_ncnt[t] += 1
        tile_ecnt[t] += degs[j]
    if (tile_ecnt > cap_e).any():
        return None
    return [np.array(t, dtype=np.int64) for t in tiles_n]


def _prepare(inputs):
    nf = np.asarray(inputs["node_feats"], np.float32)
    ef = np.asarray(inputs["edge_feats"], np.float32)
    glob = np.asarray(inputs["globals_"], np.float32)
    recv = np.asarray(inputs["receivers"]).astype(np.int64)
    ngraph = np.asarray(inputs["node_graph"]).astype(np.int64)

    cnt = np.bincount(recv, minlength=N_NODES).astype(np.int64)
    egraph = ngraph[recv]
    ncnt_g = np.bincount(ngraph, minlength=N_GRAPHS)
    ecnt_g = np.bincount(egraph, minlength=N_GRAPHS)

    # pair heavy graphs with light ones to balance nodes across cores
    order = np.argsort(ncnt_g, kind="stable")
    graph_core = np.zeros(N_GRAPHS, np.int64)
    graph_slot = np.zeros(N_GRAPHS, np.int64)
    core_graphs = []
    for c in range(N_CORES):
        ga, gb = int(order[c]), int(order[N_GRAPHS - 1 - c])
        graph_core[ga] = c
        graph_slot[ga] = 0
        graph_core[gb] = c
        graph_slot[gb] = 1
        core_graphs.append((ga, gb))

    node_core = graph_core[ngraph]
    edge_core = graph_core[egraph]

    core_nodes = [np.where(node_core == c)[0] for c in range(N_CORES)]
    NT = int(max((len(cn) + 127) // 128 for cn in core_nodes))

    packs = None
    K0 = max(1, int(max(np.bincount(edge_core, minlength=N_CORES)) + NT * 128 - 1)
             // (NT * 128))
    if K0 % 2:
        K0 += 1
    for k0 in range(K0, K0 + 13, 2):
        trial = []
        ok = True
        for c in range(N_CORES):
            p = _pack_core(core_nodes[c], cnt[core_nodes[c]], NT, k0 * 128)
            if p is None:
                ok = False
                break
            trial.append(p)
        if ok:
            packs, K0 = trial, k0
            break
    assert packs is not None, "bin packing failed"

    NPAD = NT * 128
    EPAD = NT * K0 * 128
    NPAIR = NT * K0 // 2

    # --- shared weights (core-independent parts)
    We1T = np.asarray(inputs["We1"], np.float32).T  # [32, 256]
    be1 = np.asarray(inputs["be1"], np.float32)
    be2 = np.asarray(inputs["be2"], np.float32)
    bn2 = np.asarray(inputs["bn2"], np.float32)

    We1TKb = np.zeros((128, 256), np.float32)
    We1TKb[0:32] = We1T
    We1TKb[32] = be1

    We2T = np.asarray(inputs["We2"], np.float32).T  # [256, 128]
    We2DR = np.concatenate([We2T[:128], We2T[128:]], axis=1)  # [128, 256]

    Wn1T = np.asarray(inputs["Wn1"], np.float32).T  # [64, 256]
    Wn1TK = np.zeros((128, 256), np.float32)
    Wn1TK[0:64] = Wn1T
    Wn1TK[64] = np.asarray(inputs["bn1"], np.float32)  # bias via ones-row
    bn1c = np.asarray(inputs["bn1"], np.float32).reshape(2, 128).T.copy()  # [128,2]

    Win1T = np.asarray(inputs["Win1"], np.float32).T  # [256, 256]
    Win1DR = np.zeros((128, 512), np.float32)
    for s in range(2):
        for i in range(2):
            Win1DR[:, 256 * s + 128 * i : 256 * s + 128 * i + 128] = \
                Win1T[128 * i : 128 * i + 128, 128 * s : 128 * s + 128]

    Wn2T = np.asarray(inputs["Wn2"], np.float32).T
    Wn2DR = np.concatenate([Wn2T[:128], Wn2T[128:]], axis=1)
    Win2T = np.asarray(inputs["Win2"], np.float32).T

    Wg2T = np.asarray(inputs["Wg2"], np.float32).T  # [16, 128]
    Wng2T = np.asarray(inputs["Wng2"], np.float32).T

    w_np = {
        "We1TKb": We1TKb.astype(npbf16),
        "We2DR": We2DR.astype(npbf16),
        "Wn1TK": Wn1TK.astype(npbf16),
        "bn1c": bn1c,
        "Win1DR": Win1DR.astype(npbf16),
        "Wn2DR": Wn2DR.astype(npbf16),
        "Win2f8": Win2T.astype(npbf16),
        "WgnT": np.asarray(inputs["Wgn"], np.float32).T.copy(),
        "WgeT": np.asarray(inputs["Wge"], np.float32).T.copy(),
        "WggT": np.asarray(inputs["Wgg"], np.float32).T.copy(),
        "bgr": np.asarray(inputs["bg"], np.float32)[None, :].copy(),
        "ones2": np.ones((1, 2), np.float32),
        "ident": np.eye(128, dtype=npbf16),
        "ident2": np.eye(2, dtype=np.float32),
    }

    slot_of_node = np.full(N_NODES, -1, np.int64)
    tile_of_node = np.full(N_NODES, -1, np.int64)
    in_maps = []
    for c in range(N_CORES):
        for t in range(NT):
            ids = packs[c][t]
            slot_of_node[ids] = t * 128 + np.arange(len(ids))
            tile_of_node[ids] = t

        # ---- edges: assign slots (grouped by receiver tile)
        eidx = np.where(edge_core == c)[0]
        et = tile_of_node[recv[eidx]]
        eorder = np.argsort(et, kind="stable")
        eidx = eidx[eorder]
        et = et[eorder]
        counts = np.bincount(et, minlength=NT)
        starts = np.concatenate([[0], np.cumsum(counts)[:-1]])
        off_in = np.arange(len(eidx)) - np.repeat(starts, counts)
        dst = et * (K0 * 128) + off_in
        assert (counts <= K0 * 128).all()

        eg_loc = graph_slot[egraph[eidx]]
        # eftM: one [128,128] column-block per chunk.
        # rows 0:32 feats, 32 ones, 33 isg0, 34 isg1, rest zero.
        eftM = np.zeros((128, EPAD), np.float32)
        eftM[0:32, dst] = ef[eidx].T
        eftM[32, dst] = 1.0
        eftM[33, dst] = (eg_loc == 0)
        eftM[34, dst] = (eg_loc == 1)

        # one-hot selectors, bf16, chunk-major: ohb[e, chunk*128 + n]
        sel = np.full(EPAD, -1, np.int64)
        sel[dst] = slot_of_node[recv[eidx]] % 128
        oh = np.zeros((EPAD, 128), np.float32)
        vmask = sel >= 0
        oh[np.where(vmask)[0], sel[vmask]] = 1.0
        oh2 = (
            oh.reshape(NT * K0, 128, 128)
            .transpose(1, 0, 2)
            .reshape(128, EPAD)
        )

        # merged e1 + e2-init stationary weights (per-core globals)
        ga, gb = core_graphs[c]
        gl = np.stack([glob[ga], glob[gb]])  # [2, 16]
        gp = gl @ Wg2T  # [2, 128]
        We1Kx = np.zeros((128, 384), np.float32)
        We1Kx[0:32, 0:256] = We1T
        We1Kx[32, 0:256] = be1
        We1Kx[32, 256:384] = be2
        We1Kx[33, 256:384] = gp[0]
        We1Kx[34, 256:384] = gp[1]

        gn = gl @ Wng2T
        gnaugK = np.zeros((128, 128), np.float32)
        gnaugK[0:2] = gn
        gnaugK[2] = bn2

        # ---- nodes
        slot_node = np.full(NPAD, -1, np.int64)
        for t in range(NT):
            ids = packs[c][t]
            slot_node[t * 128 : t * 128 + len(ids)] = ids
        valid = slot_node >= 0
        sn = np.where(valid, slot_node, 0)

        nftK = np.zeros((128, NPAD), np.float32)
        nftK[0:64][:, valid] = nf[sn[valid]].T
        nftK[64] = valid * 1.0  # ones-row pairs with the bn1 row in Wn1TK

        ng_loc = graph_slot[ngraph[sn]]
        nhotK = np.zeros((128, NPAD), np.float32)
        nhotK[0] = valid * (ng_loc == 0)
        nhotK[1] = valid * (ng_loc == 1)
        nhotK[2] = valid * 1.0

        invc2 = np.zeros((NPAD, 1), np.float32)
        invc2[valid, 0] = 1.0 / np.maximum(cnt[sn[valid]], 1)
        invc2 = invc2.reshape(NT, 128).T.copy()  # [128, NT]

        # zero-padded pool weight stationaries: cols 0:2 carry the weights
        poolw2 = np.zeros((NPAD, 256), np.float32)
        for g in range(GPC):
            gid = core_graphs[c][g]
            m = valid & (ng_loc == g)
            poolw2[m, g] = 1.0 / max(ncnt_g[gid], 1)
            poolw2[m, 128 + g] = cnt[sn[m]] / max(ecnt_g[gid], 1)

        globT = gl.T.copy()  # [16, 2]

        m = {
            "eftM": eftM.astype(npbf16),
            "oh2": oh2.astype(npbf16),
            "We1Kx": We1Kx.astype(npbf16),
            "gnaugK": gnaugK.astype(npbf16),
            "nftK": nftK.astype(npbf16),
            "nhotK": nhotK.astype(npbf16),
            "invc2": invc2,
            "poolw2": poolw2.astype(npbf16),
            "globT": globT,
        }
        m.update(w_np)
        in_maps.append(m)

    return in_maps, NT, K0, [core_graphs[c] for c in range(N_CORES)]


# ---------------------------------------------------------------------------
# Device program (identical on all cores)


def _build(NT, K0):
    Relu = mybir.ActivationFunctionType.Relu
    Copy = mybir.ActivationFunctionType.Copy

    nc = bass.Bass()
    NPAD = NT * 128
    EPAD = NT * K0 * 128
    NPAIR = NT * K0 // 2
    PPT = K0 // 2  # pairs per tile
    CW = K0 * 128  # eftM cols per tile
    OW = PPT * 256  # oh2 cols per tile

    d_eftM = nc.dram_tensor("eftM", [128, EPAD], BF16, kind="ExternalInput")
    d_oh2 = nc.dram_tensor("oh2", [128, EPAD], BF16, kind="ExternalInput")
    d_We1Kx = nc.dram_tensor("We1Kx", [128, 384], BF16, kind="ExternalInput")
    d_gnaugK = nc.dram_tensor("gnaugK", [128, 128], BF16, kind="ExternalInput")
    d_nftK = nc.dram_tensor("nftK", [128, NPAD], BF16, kind="ExternalInput")
    d_nhotK = nc.dram_tensor("nhotK", [128, NPAD], BF16, kind="ExternalInput")
    d_invc2 = nc.dram_tensor("invc2", [128, NT], F32, kind="ExternalInput")
    d_poolw2 = nc.dram_tensor("poolw2", [NPAD, 256], BF16, kind="ExternalInput")
    d_globT = nc.dram_tensor("globT", [16, 2], F32, kind="ExternalInput")

    d_We1TKb = nc.dram_tensor("We1TKb", [128, 256], BF16, kind="ExternalInput")
    d_We2DR = nc.dram_tensor("We2DR", [128, 256], BF16, kind="ExternalInput")
    d_Wn1TK = nc.dram_tensor("Wn1TK", [128, 256], BF16, kind="ExternalInput")
    d_bn1c = nc.dram_tensor("bn1c", [128, 2], F32, kind="ExternalInput")
    d_Win1DR = nc.dram_tensor("Win1DR", [128, 512], BF16, kind="ExternalInput")
    d_Wn2DR = nc.dram_tensor("Wn2DR", [128, 256], BF16, kind="ExternalInput")
    d_Win2f8 = nc.dram_tensor("Win2f8", [128, 128], BF16, kind="ExternalInput")
    d_WgnT = nc.dram_tensor("WgnT", [128, 128], F32, kind="ExternalInput")
    d_WgeT = nc.dram_tensor("WgeT", [128, 128], F32, kind="ExternalInput")
    d_WggT = nc.dram_tensor("WggT", [16, 128], F32, kind="ExternalInput")
    d_bgr = nc.dram_tensor("bgr", [1, 128], F32, kind="ExternalInput")
    d_ones2 = nc.dram_tensor("ones2", [1, 2], F32, kind="ExternalInput")
    d_ident = nc.dram_tensor("ident", [128, 128], BF16, kind="ExternalInput")
    d_ident2 = nc.dram_tensor("ident2", [2, 2], F32, kind="ExternalInput")

    d_out = nc.dram_tensor("out", [128, 2], F32, kind="ExternalOutput")

    def r3(ap, blk):
        return ap.rearrange("p (a b) -> p a b", a=2, b=blk)

    with tile.TileContext(nc) as tc:
        with tc.tile_pool(name="wp", bufs=1) as wp:
            def wtile(dram, shape, dt):
                t = wp.tile(shape, dt, tag=dram.name)
                nc.sync.dma_start(t[:], dram[:])
                return t

            We1Kx = wtile(d_We1Kx, [128, 384], BF16)
            We1TKb = wtile(d_We1TKb, [128, 256], BF16)
            We2DR = wtile(d_We2DR, [128, 256], BF16)
            gnaugK = wtile(d_gnaugK, [128, 128], BF16)
            Wn1TK = wtile(d_Wn1TK, [128, 256], BF16)
            bn1c = wtile(d_bn1c, [128, 2], F32)
            Win1DR = wtile(d_Win1DR, [128, 512], BF16)
            Wn2DR = wtile(d_Wn2DR, [128, 256], BF16)
            Win2f8 = wtile(d_Win2f8, [128, 128], BF16)
            WgnT = wtile(d_WgnT, [128, 128], F32)
            WgeT = wtile(d_WgeT, [128, 128], F32)
            WggT = wtile(d_WggT, [16, 128], F32)
            bgr = wtile(d_bgr, [1, 128], F32)
            ones2 = wtile(d_ones2, [1, 2], F32)
            ident = wtile(d_ident, [128, 128], BF16)
            ident2 = wtile(d_ident2, [2, 2], F32)
            globT = wtile(d_globT, [16, 2], F32)
            invc2 = wtile(d_invc2, [128, NT], F32)

            with tc.tile_pool(name="ep", bufs=3) as ep, \
                 tc.tile_pool(name="esb", bufs=4) as esb, \
                 tc.tile_pool(name="nsb", bufs=3) as nsb, \
                 tc.tile_pool(name="psME", bufs=2, space=bass.MemorySpace.PSUM) as psME, \
                 tc.tile_pool(name="psMO", bufs=2, space=bass.MemorySpace.PSUM) as psMO, \
                 tc.tile_pool(name="psT1", bufs=1, space=bass.MemorySpace.PSUM) as psT1, \
                 tc.tile_pool(name="psAgg", bufs=1, space=bass.MemorySpace.PSUM) as psAgg, \
                 tc.tile_pool(name="psA", bufs=1, space=bass.MemorySpace.PSUM) as psA, \
                 tc.tile_pool(name="psB", bufs=1, space=bass.MemorySpace.PSUM) as psB:

                # node-phase PSUM shares two banks; all groups are closed
                # per tile, so interleavings stay legal.
                bankA = psA.tile([128, 512], F32, tag="bankA")
                pn1 = bankA[:, 0:256]
                bankB = psB.tile([128, 512], F32, tag="bankB")
                ptrT = bankB[:, 0:384]
                pn2 = bankB[:, 384:512]
                accP = None

                state = {"pair": None, "node": None, "accP": None}

                def emit_pair_tail(p):
                    """pe2 + evacuations + aggregation for a pair."""
                    mgE, mgO, e1bf, ef2, pagg, e0, e1s, oht, j, ppt_ = p[:10]
                    nc.tensor.matmul(mgE[:, 256:384], e1bf[:, 0:128],
                                     We2DR[:, 0:128], start=False, stop=False)
                    nc.tensor.matmul(mgE[:, 256:384], e1bf[:, 128:256],
                                     We2DR[:, 128:256], start=False, stop=True)
                    nc.tensor.matmul(mgO[:, 256:384], e1bf[:, 256:384],
                                     We2DR[:, 0:128], start=False, stop=False)
                    nc.tensor.matmul(mgO[:, 256:384], e1bf[:, 384:512],
                                     We2DR[:, 128:256], start=False, stop=True)
                    # one full-width relu evac per chunk (DVE)
                    nc.vector.tensor_scalar_max(ef2[:, 0:384], mgE[:], 0.0)
                    nc.vector.tensor_scalar_max(ef2[:, 384:768], mgO[:], 0.0)
                    nc.tensor.matmul(pagg[:], oht[:, e0], ef2[:, 0:384],
                                     start=(j == 0), stop=False)
                    nc.tensor.matmul(pagg[:], oht[:, e1s], ef2[:, 384:768],
                                     start=False, stop=(j == ppt_ - 1))

                def emit_node_rest(nd):
                    """node phase after aggsb: transposes, n1, n2, pools."""
                    aggsb, nftt, nht, pw = nd
                    nc.tensor.matmul(ptrT[:, 0:128], aggsb[:, 0:128], ident[:],
                                     start=True, stop=True)
                    nc.tensor.matmul(ptrT[:, 128:256], aggsb[:, 128:256], ident[:],
                                     start=True, stop=True)
                    nc.tensor.matmul(ptrT[:, 256:384], aggsb[:, 256:384], ident[:],
                                     start=True, stop=True)
                    aggT = nsb.tile([128, 384], BF16, tag="aggT")
                    nc.vector.tensor_copy(aggT[:], ptrT)

                    for s in range(2):
                        sc = slice(s * 128, (s + 1) * 128)
                        nc.tensor.matmul(pn1[:, sc], Wn1TK[:, sc], nftt[:],
                                         start=True, stop=False)
                        nc.tensor.matmul(pn1[:, sc],
                                         Win1DR[:, 256 * s : 256 * s + 128],
                                         aggT[:, 0:128], start=False, stop=False)
                        nc.tensor.matmul(pn1[:, sc],
                                         Win1DR[:, 256 * s + 128 : 256 * s + 256],
                                         aggT[:, 128:256], start=False, stop=True)
                    n1bf = nsb.tile([128, 256], BF16, tag="n1bf")
                    nc.scalar.activation(n1bf[:], pn1, Relu)

                    nc.tensor.matmul(pn2, nht[:], gnaugK[:], start=True, stop=False)
                    nc.tensor.matmul(pn2, n1bf[:, 0:128], Wn2DR[:, 0:128],
                                     start=False, stop=False)
                    nc.tensor.matmul(pn2, n1bf[:, 128:256], Wn2DR[:, 128:256],
                                     start=False, stop=False)
                    nc.tensor.matmul(pn2, aggT[:, 256:384], Win2f8[:],
                                     start=False, stop=True)
                    n2bf = nsb.tile([128, 128], BF16, tag="n2bf")
                    nc.vector.tensor_scalar_max(n2bf[:], pn2, 0.0)

                    ppt = bankA[:, 256:512]
                    nc.tensor.matmul(ppt[:, 0:128], pw[:, 0:128], n2bf[:],
                                     start=True, stop=True)
                    nc.tensor.matmul(ppt[:, 128:256], pw[:, 128:256],
                                     aggsb[:, 256:384], start=True, stop=True)
                    accP_new = nsb.tile([2, 256], F32, tag="accP")
                    if state["accP"] is None:
                        nc.vector.tensor_copy(accP_new[:], ppt[0:2, :])
                    else:
                        nc.vector.tensor_tensor(accP_new[:], state["accP"][:],
                                                ppt[0:2, :],
                                                op=mybir.AluOpType.add)
                    state["accP"] = accP_new

                for t in range(NT):
                    eftt = ep.tile([128, CW], BF16, tag="eftt")
                    nc.sync.dma_start(eftt[:], d_eftM[:, t * CW : (t + 1) * CW])
                    oht = ep.tile([128, CW], BF16, tag="oht")
                    nc.sync.dma_start(oht[:], d_oh2[:, t * CW : (t + 1) * CW])
                    nftt = ep.tile([128, 128], BF16, tag="nftt")
                    nc.sync.dma_start(nftt[:], d_nftK[:, t * 128 : (t + 1) * 128])
                    nht = ep.tile([128, 128], BF16, tag="nht")
                    nc.sync.dma_start(nht[:], d_nhotK[:, t * 128 : (t + 1) * 128])
                    pw = ep.tile([128, 256], BF16, tag="pw")
                    nc.sync.dma_start(pw[:], d_poolw2[t * 128 : (t + 1) * 128, :])

                    pagg = psAgg.tile([128, 384], F32, tag="pagg")

                    for j in range(PPT):
                        e0 = slice(2 * j * 128, 2 * j * 128 + 128)
                        e1s = slice((2 * j + 1) * 128, (2 * j + 1) * 128 + 128)
                        epr = slice(2 * j * 128, 2 * j * 128 + 256)

                        mgE = psME.tile([128, 384], F32, tag="mgE")
                        mgO = psMO.tile([128, 384], F32, tag="mgO")
                        e1T2 = psT1.tile([128, 512], F32, tag="e1T2")

                        # merged e1 + e2-init (FD=384), one per chunk
                        nc.tensor.matmul(mgE[:], eftt[:, e0], We1Kx[:],
                                         start=True, stop=False)
                        nc.tensor.matmul(mgO[:], eftt[:, e1s], We1Kx[:],
                                         start=True, stop=False)
                        # e1T blocks for the pair (FD=256 each)
                        nc.tensor.matmul(e1T2[:, 0:256], We1TKb[:, 0:128],
                                         eftt[:, epr], start=True, stop=True)
                        nc.tensor.matmul(e1T2[:, 256:512], We1TKb[:, 128:256],
                                         eftt[:, epr], start=True, stop=True)

                        ef2 = esb.tile([128, 768], BF16, tag="ef2")
                        e1bf = esb.tile([128, 512], BF16, tag="e1bf")

                        # software pipeline: finish the PREVIOUS pair now that
                        # this pair's first-stage matmuls are in the stream.
                        # (aggsb is emitted before this pair's e1bf so the Act
                        # queue unblocks the next tile's pagg group promptly.)
                        if state["pair"] is not None:
                            pt = state["pair"]
                            emit_pair_tail(pt)
                            if pt[10] != t:
                                # that was the previous tile's last pair:
                                # kick off its node phase
                                aggsb = nsb.tile([128, 384], BF16, tag="aggsb")
                                nc.scalar.activation(
                                    aggsb[:], pt[4][:], Copy,
                                    scale=invc2[:, pt[10] : pt[10] + 1])
                                state["node"] = (aggsb, pt[11], pt[12], pt[13])

                        # evacuate e1 feat-major -> e1bf (Act: relu, reshuffle
                        # blk-major -> chunk-major while casting)
                        src = e1T2[:].rearrange("p (b c e) -> p c b e", b=2, c=2, e=128)
                        dst = e1bf[:].rearrange("p (c b e) -> p c b e", c=2, b=2, e=128)
                        nc.scalar.activation(dst, src, Relu)

                        state["pair"] = (mgE, mgO, e1bf, ef2, pagg, e0, e1s,
                                         oht, j, PPT, t, nftt, nht, pw)

                        # deferred node phase of the previous tile
                        if j == 2 and state["node"] is not None:
                            emit_node_rest(state["node"])
                            state["node"] = None

                # drain: last pair + last node phase
                pt = state["pair"]
                emit_pair_tail(pt)
                aggsb = nsb.tile([128, 384], BF16, tag="aggsb")
                nc.scalar.activation(aggsb[:], pt[4][:], Copy,
                                     scale=invc2[:, pt[10] : pt[10] + 1])
                emit_node_rest((aggsb, pt[11], pt[12], pt[13]))
                accP = state["accP"]

                # ----------------- final projection -----------------
                ptr2 = bankB[:, 0:4]
                nc.tensor.matmul(ptr2[:, 0:2], accP[:, 0:128], ident2[:],
                                 start=True, stop=True)
                nc.tensor.matmul(ptr2[:, 2:4], accP[:, 128:256], ident2[:],
                                 start=True, stop=True)
                nt2 = nsb.tile([128, 4], F32, tag="nt2")
                nc.scalar.activation(nt2[:], ptr2, Copy)

                pout = bankA[:, 256:258]
                nc.tensor.matmul(pout, WgnT[:], nt2[:, 0:2], start=True, stop=False)
                nc.tensor.matmul(pout, WgeT[:], nt2[:, 2:4], start=False, stop=False)
                nc.tensor.matmul(pout, WggT[:], globT[:], start=False, stop=False)
                nc.tensor.matmul(pout, bgr[:], ones2[:], start=False, stop=True)
                outsb = nsb.tile([128, 2], F32, tag="outsb")
                nc.scalar.activation(outsb[:], pout, Copy)
                nc.sync.dma_start(d_out[:], outsb[:])

    return nc


_CACHE = {}


def _get_nc(NT, K0):
    key = (NT, K0)
    if key not in _CACHE:
        _CACHE[key] = _build(NT, K0)
    return _CACHE[key]


def _run(inputs, trace=False):
    in_maps, NT, K0, core_graphs = _prepare(inputs)
    nc = _get_nc(NT, K0)
    res = run_bass_kernel_spmd(nc, in_maps, list(range(N_CORES)), trace=trace)
    out = np.zeros((N_GRAPHS, 128), np.float32)
    for c in range(N_CORES):
        r = np.asarray(res.results[c]["out"], np.float32)
        ga, gb = core_graphs[c]
        out[ga] = r[:, 0]
        out[gb] = r[:, 1]
    return out, res


def kernel(**inputs):
    out, _ = _run(inputs, trace=False)
    return out


def kernel_traced(**inputs):
    return _run(inputs, trace=True)
